# revision 1
# baseline (speedup 1.0000x reference)
"""Trainium2 Bass kernel for nn_Attention_57080115364834.

Reference computation (B=4, C=512, H=W=64, N=H*W=4096 tokens):
    t = x.reshape(b, c, n).swapaxes(1, 2)          # (b, n, c)
    q, k, v = t@Wq.T+bq, t@Wk.T+bk, t@Wv.T+bv
    attn = softmax(q @ k.T / sqrt(c))              # (b, n, n)
    out = (attn @ v) @ Wo.T + bo                   # (b, n, c)
    return out.reshape(b, c, h, w)                 # raw view, no permute

Sharding: 8 cores = 4 batches x 2 query-halves. Each core holds the full
x[b] (C x N, which is exactly t.T - the natural layout for Trainium
matmuls) so it computes its batch's full K^T (c,n) and VW (n,c) locally,
plus Q^T for its 2048-token half. No collectives.

Host-side algebra folds both post-attention linear steps away:
  - softmax rows sum to 1  =>  attn @ (v+bv) == attn@v + bv, so the v
    bias becomes an output bias  bo' = Wo @ bv + bo.
  - (attn @ v) @ Wo.T == attn @ (v @ Wo.T) == attn @ (t @ (Wo@Wv).T),
    so with Wvo = Wo@Wv precomputed on host, the VW projection directly
    produces final-channel values and no device-side output projection
    is needed.
The kernel returns outT (c, n) per core; the host transposes during
unsharding (a pure layout move).

Per-core dataflow (main matmuls bf16 with f32 PSUM accumulation; the
softmax normalization chain runs in f32/f32r, so 1/rowsum is exact):
  kT[c,m]   = Wk @ tC + bk   (lhsT=WkT chunk, rhs=tC chunk; bias on ACT evac)
  VW[m,c]   = tC.T @ WvoT    (lhsT=tC chunk,  rhs=WvoT)
  qT[c,n]   = Wq @ tCq + bq  per 512-token n-chunk
  ST[m,n]   = kT.T-chunks @ qT       (scores, transposed)
  P[m,n]    = exp(ST/sqrt(c))        ScalarE, no max-subtract (|scores|<~2)
  acc[m%128,n] += P                  DVE accumulate (for rowsum)
  OT[c,n]  += VW-chunk.T @ P         (PSUM-accumulated over m-tiles)
  OT[c,n]  += bo'[c-chunk] x rowsum[n]   (K=1 matmul; exact bias)
  rowsum[1,n] = ones.T @ acc (f32r MM); rinv broadcast via K=1 MM
  outT[c,n] = OT * rinv_bc           (DVE, PSUM->SBUF) -> DMA

Chunk tails are software-pipelined: chunk nb's rowsum/normalize/store is
emitted after chunk nb+1's q-projection so the scheduler never stalls
the TensorEngine on the rowsum chain at chunk boundaries.
"""

import sys

for _p in ("/opt/trn_rl_repo", "/root/.axon_site/_ro/trn_rl_repo"):
    if _p not in sys.path:
        sys.path.append(_p)

import numpy as np
import ml_dtypes

import concourse.bacc as bacc
import concourse.mybir as mybir
import concourse.tile as tile
from concourse.bass_utils import run_bass_kernel_spmd

DT = mybir.dt.float32
FR = mybir.dt.float32r
BF = mybir.dt.bfloat16
AFT = mybir.ActivationFunctionType

B, C, HW = 4, 512, 4096          # batch, channels, tokens per batch
NQ = HW // 2                     # q tokens per core (2048)
CK = C // 128                    # contraction chunks (4)
MT = HW // 128                   # key/value tiles (32)
NB = NQ // 512                   # q-chunks per core (4)
SCALE = 1.0 / float(np.sqrt(C))
N_CORES = 8

_compiled = None
_ONES = np.ones(128, dtype=np.float32)


def _build():
    nc = bacc.Bacc("TRN2", target_bir_lowering=False)

    xt_e = nc.declare_dram_parameter("xt", [C, HW], BF, isOutput=False)
    xq_e = nc.declare_dram_parameter("xq", [C, NQ], BF, isOutput=False)
    wqt_e = nc.declare_dram_parameter("wqt", [C, C], BF, isOutput=False)
    wkt_e = nc.declare_dram_parameter("wkt", [C, C], BF, isOutput=False)
    wvot_e = nc.declare_dram_parameter("wvot", [C, C], BF, isOutput=False)
    bq_e = nc.declare_dram_parameter("bq", [C], DT, isOutput=False)
    bk_e = nc.declare_dram_parameter("bk", [C], DT, isOutput=False)
    bop_e = nc.declare_dram_parameter("bop", [C], BF, isOutput=False)
    ones_fr_e = nc.declare_dram_parameter("ones_fr", [128], FR, isOutput=False)
    out_e = nc.declare_dram_parameter("outT", [C, NQ], DT, isOutput=True)

    with tile.TileContext(nc) as tc:
        with (
            tc.tile_pool(name="kt", bufs=1) as kt_pool,
            tc.tile_pool(name="vv", bufs=1) as vv_pool,
            tc.tile_pool(name="wq", bufs=1) as wq_pool,
            tc.tile_pool(name="consts", bufs=1) as c_pool,
        ):
            # ---- persistent tiles (phase-2-only DMAs emitted late so they
            # don't delay the first phase-1 matmul) ----
            kt_sb = [kt_pool.tile([128, HW], BF, tag=f"k{i}", name=f"k{i}") for i in range(CK)]
            vw_sb = [vv_pool.tile([128, C], BF, tag=f"v{i}", name=f"v{i}") for i in range(MT)]
            wq_sb = [wq_pool.tile([128, C], BF, tag=f"wq{i}", name=f"wq{i}") for i in range(CK)]

            bq_t = c_pool.tile([128, CK], DT, tag="bq", name="bq_t")
            bk_t = c_pool.tile([128, CK], DT, tag="bk", name="bk_t")
            bop_row = c_pool.tile([1, C], BF, tag="bop", name="bop_row")
            ones_col_r = c_pool.tile([128, 1], FR, tag="onescr", name="ones_col_r")
            ones_row_r = c_pool.tile([1, 128], FR, tag="onesrr", name="ones_row_r")
            for t in range(CK):
                nc.sync.dma_start(bk_t[:, t:t + 1], bk_e[t * 128:(t + 1) * 128])
            nc.sync.dma_start(ones_col_r[:, 0:1], ones_fr_e[:])
            nc.sync.dma_start(ones_row_r[0:1, :], ones_fr_e[:])

            # ---- phase 1: kT (c,m) and VW (m,c) projections ----
            with (
                tc.tile_pool(name="wkv", bufs=1) as wkv_pool,
                tc.tile_pool(name="tcc", bufs=3) as tcc_pool,
                tc.tile_pool(name="ps1", bufs=2, space="PSUM") as ps1,
            ):
                wk_sb = [wkv_pool.tile([128, C], BF, tag=f"wk{i}", name=f"wk{i}") for i in range(CK)]
                wv_sb = [wkv_pool.tile([128, C], BF, tag=f"wv{i}", name=f"wv{i}") for i in range(CK)]
                for i in range(CK):
                    nc.sync.dma_start(wk_sb[i][:], wkt_e[i * 128:(i + 1) * 128, :])
                for i in range(CK):
                    nc.sync.dma_start(wv_sb[i][:], wvot_e[i * 128:(i + 1) * 128, :])

                for j in range(HW // 512):
                    tcs = [tcc_pool.tile([128, 512], BF, tag=f"tc{ci}", name=f"tc{ci}") for ci in range(CK)]
                    for ci in range(CK):
                        nc.gpsimd.dma_start(
                            tcs[ci][:], xt_e[ci * 128:(ci + 1) * 128, j * 512:(j + 1) * 512]
                        )
                    # kT token-chunk j, all four output-channel chunks
                    for co in range(CK):
                        pk = ps1.tile([128, 512], DT, tag="pk", name="pk")
                        for ci in range(CK):
                            nc.tensor.matmul(
                                pk[:], wk_sb[ci][:, co * 128:(co + 1) * 128],
                                tcs[ci][:], start=(ci == 0), stop=(ci == CK - 1),
                            )
                        nc.scalar.activation(
                            kt_sb[co][:, j * 512:(j + 1) * 512], pk[:], AFT.Identity,
                            bias=bk_t[:, co:co + 1],
                        )
                    # VW m-tiles 4j..4j+3 (no bias: folded into bo')
                    for ml in range(4):
                        pv = ps1.tile([128, 512], DT, tag="pv", name="pv")
                        for ci in range(CK):
                            nc.tensor.matmul(
                                pv[:], tcs[ci][:, ml * 128:(ml + 1) * 128],
                                wv_sb[ci][:], start=(ci == 0), stop=(ci == CK - 1),
                            )
                        nc.vector.tensor_copy(vw_sb[4 * j + ml][:], pv[:])

            # phase-2 weights/consts arrive while phase-1 compute runs
            for i in range(CK):
                nc.sync.dma_start(wq_sb[i][:], wqt_e[i * 128:(i + 1) * 128, :])
            for t in range(CK):
                nc.sync.dma_start(bq_t[:, t:t + 1], bq_e[t * 128:(t + 1) * 128])
            nc.sync.dma_start(bop_row[0:1, :], bop_e[:])

            # ---- phase 2: attention per 512-token q-chunk ----
            with (
                tc.tile_pool(name="xqp", bufs=2) as xq_pool,
                tc.tile_pool(name="qcp", bufs=2) as qc_pool,
                tc.tile_pool(name="pexp", bufs=6) as pe_pool,
                tc.tile_pool(name="accp", bufs=2) as acc_pool,
                tc.tile_pool(name="rsp", bufs=2) as rs_pool,
                tc.tile_pool(name="outp", bufs=3) as out_pool,
                tc.tile_pool(name="smallp", bufs=2) as small_pool,
                tc.tile_pool(name="ps2", bufs=4, space="PSUM") as ps2,
                tc.tile_pool(name="psot", bufs=1, space="PSUM") as psot,
            ):
                def emit_tail(tnb, acc, ots):
                    # rowsum via one f32r ones-matmul; reciprocal row;
                    # broadcast via K=1 matmul; exact bias; normalize + store
                    rs = ps2.tile([1, 512], DT, tag="st", name="rs")
                    nc.tensor.matmul(rs[:], ones_col_r[:, 0:1], acc[:], start=True, stop=True)
                    rs_row = small_pool.tile([1, 512], BF, tag="rsrow", name="rs_row")
                    nc.scalar.activation(rs_row[:], rs[:], AFT.Copy)
                    rinv_row = small_pool.tile([1, 512], FR, tag="rinvrow", name="rinv_row")
                    with nc.allow_low_precision(reason="f32r stores full f32 bits; PE rounds on read"):
                        nc.vector.reciprocal(rinv_row[:], rs[:])
                    rbc_ps = ps2.tile([128, 512], DT, tag="st", name="rbc_ps")
                    nc.tensor.matmul(rbc_ps[:], ones_row_r[0:1, :], rinv_row[0:1, :],
                                     start=True, stop=True)
                    rinv_bc = rs_pool.tile([128, 512], DT, tag="rinvbc", name="rinv_bc")
                    nc.vector.tensor_copy(rinv_bc[:], rbc_ps[:])
                    for co in range(CK):
                        nc.tensor.matmul(
                            ots[co][:], bop_row[0:1, co * 128:(co + 1) * 128],
                            rs_row[0:1, :], start=False, stop=True, skip_group_check=True,
                        )
                        oc = out_pool.tile([128, 512], DT, tag="oc", name="oc", bufs=5)
                        nc.vector.tensor_mul(oc[:], ots[co][:], rinv_bc[:])
                        nc.sync.dma_start(
                            out_e[co * 128:(co + 1) * 128, tnb * 512:(tnb + 1) * 512], oc[:]
                        )

                prev = None
                for nb in range(NB):
                    xqs = [xq_pool.tile([128, 512], BF, tag=f"xq{ci}", name=f"xq{ci}") for ci in range(CK)]
                    for ci in range(CK):
                        nc.gpsimd.dma_start(
                            xqs[ci][:], xq_e[ci * 128:(ci + 1) * 128, nb * 512:(nb + 1) * 512]
                        )
                    # qT chunk (c, 512)
                    qcs = []
                    for co in range(CK):
                        pq = ps2.tile([128, 512], DT, tag="st", name="st")
                        for ci in range(CK):
                            nc.tensor.matmul(
                                pq[:], wq_sb[ci][:, co * 128:(co + 1) * 128],
                                xqs[ci][:], start=(ci == 0), stop=(ci == CK - 1),
                            )
                        qc = qc_pool.tile([128, 512], BF, tag=f"qc{co}", name=f"qc{co}")
                        nc.scalar.activation(qc[:], pq[:], AFT.Identity, bias=bq_t[:, co:co + 1])
                        qcs.append(qc)

                    # previous chunk's tail is emitted here so its rowsum chain
                    # never blocks this chunk's q-projection in the ACT queue
                    if prev is not None:
                        emit_tail(*prev)

                    acc = acc_pool.tile([128, 512], FR, tag="acc", name="acc")
                    ots = [psot.tile([128, 512], DT, tag=f"ot{co}", name=f"ot{co}") for co in range(CK)]
                    for mt in range(MT):
                        st = ps2.tile([128, 512], DT, tag="st", name="st")
                        for ci in range(CK):
                            nc.tensor.matmul(
                                st[:], kt_sb[ci][:, mt * 128:(mt + 1) * 128],
                                qcs[ci][:], start=(ci == 0), stop=(ci == CK - 1),
                            )
                        pexp = pe_pool.tile([128, 512], BF, tag="pe", name="pexp")
                        nc.scalar.activation(pexp[:], st[:], AFT.Exp, scale=SCALE)
                        if mt == 0:
                            nc.vector.tensor_copy(acc[:], pexp[:])
                        else:
                            nc.vector.tensor_add(acc[:], acc[:], pexp[:])
                        for co in range(CK):
                            nc.tensor.matmul(
                                ots[co][:], vw_sb[mt][:, co * 128:(co + 1) * 128],
                                pexp[:],
                                start=(mt == 0), stop=False, skip_group_check=True,
                            )
                    prev = (nb, acc, ots)

                emit_tail(*prev)

    nc.compile()
    return nc


def _get_compiled():
    global _compiled
    if _compiled is None:
        _compiled = _build()
    return _compiled


def kernel(**inputs):
    x = np.ascontiguousarray(np.asarray(inputs["x"], dtype=np.float32))
    wq = np.asarray(inputs["Wq"], dtype=np.float32)
    wk = np.asarray(inputs["Wk"], dtype=np.float32)
    wv = np.asarray(inputs["Wv"], dtype=np.float32)
    wo = np.asarray(inputs["Wo"], dtype=np.float32)
    bq = np.ascontiguousarray(np.asarray(inputs["bq"], dtype=np.float32))
    bk = np.ascontiguousarray(np.asarray(inputs["bk"], dtype=np.float32))
    bv = np.asarray(inputs["bv"], dtype=np.float32)
    bo = np.asarray(inputs["bo"], dtype=np.float32)

    wqt = np.ascontiguousarray(wq.T.astype(ml_dtypes.bfloat16))
    wkt = np.ascontiguousarray(wk.T.astype(ml_dtypes.bfloat16))
    wvot = np.ascontiguousarray((wo @ wv).T.astype(ml_dtypes.bfloat16))
    bop = np.ascontiguousarray((wo @ bv + bo).astype(ml_dtypes.bfloat16))

    xb = x.reshape(B, C, HW).astype(ml_dtypes.bfloat16)
    in_maps = []
    for core in range(N_CORES):
        bi, h = core // 2, core % 2
        in_maps.append({
            "xt": np.ascontiguousarray(xb[bi]),
            "xq": np.ascontiguousarray(xb[bi][:, h * NQ:(h + 1) * NQ]),
            "wqt": wqt, "wkt": wkt, "wvot": wvot,
            "bq": bq, "bk": bk, "bop": bop, "ones_fr": _ONES,
        })

    nc = _get_compiled()
    res = run_bass_kernel_spmd(nc, in_maps, core_ids=list(range(N_CORES)))

    out = np.empty((B, HW, C), dtype=np.float32)
    for core in range(N_CORES):
        bi, h = core // 2, core % 2
        out[bi, h * NQ:(h + 1) * NQ, :] = res.results[core]["outT"].T
    return out.reshape(B, C, 64, 64)



# revision 5
# speedup vs baseline: 1.2752x; 1.2752x over previous
"""Trainium2 Bass kernel for nn_Attention_57080115364834.

Reference computation (B=4, C=512, H=W=64, N=H*W=4096 tokens):
    t = x.reshape(b, c, n).swapaxes(1, 2)          # (b, n, c)
    q, k, v = t@Wq.T+bq, t@Wk.T+bk, t@Wv.T+bv
    attn = softmax(q @ k.T / sqrt(c))              # (b, n, n)
    out = (attn @ v) @ Wo.T + bo                   # (b, n, c)
    return out.reshape(b, c, h, w)                 # raw view, no permute

Sharding: 8 cores = 4 batches x 2 query-halves, no collectives.

Host-side algebra removes BOTH weight applications from the key/value
token streams so no projection ever runs over the full 4096-token axis:
  - scores = (t Wq^T)(t Wk^T)^T = t A t^T with A = Wq^T Wk precomputed
    on host.  The device projects only the queries (q' = t A) and uses
    RAW x as the key matrix - the whole K projection disappears.
  - (attn @ v) @ Wo^T = (attn @ t) @ (Wo Wv)^T.  The device contracts
    P against raw x (U = P t, same cost as P @ v), then projects
    U Wvo^T over the core's 2048 queries only - half the cost of
    projecting v over all 4096 tokens, and no work is duplicated
    between the two cores sharing a batch.
  - bk shifts every score in a row n by the same amount (q_n . bk), so
    softmax cancels it exactly: dropped.  bq contributes scale*(bq Wk
    t^T), a per-key row precomputed on host and applied through the
    (otherwise free) bias operand of the Exp activation.  bv/bo fold to
    bo' = Wo bv + bo, applied via K=1 matmuls only when nonzero (the
    compiled variant is keyed on that flag).

Per-core dataflow (matmuls bf16 with f32 PSUM; normalization in f32):
  q'T[c,n]  = A^T-chunks @ tC-chunk     per 512-query chunk (16 MMs)
  ST[m,n]   = tC-chunks @ q'T           (scores, keys = raw x)
  P[m,n]    = exp(ST*scale + sbias)     ScalarE
  acc      += P                         DVE (rowsum accumulate)
  UT[c',n] += xN-chunk.T @ P            PSUM-accumulated over m-tiles
  u[c',n]   = UT evac (bf16, ScalarE)   - no rowsum dependency
  OT[c,n]   = WvoT-chunks @ u           (16 MMs)
  rowsum    = ones.T @ acc (f32r MM); broadcast via K=1 MM;
              rinv = reciprocal_approx_fast (DVE, 128-wide)
  outT[c,n] = OT * rinv                 (DVE, PSUM->SBUF) -> DMA

The previous chunk's rowsum chain and U-projection are emitted between
the next chunk's q'-projection and its scores loop, so the PE never
waits on the ACT/DVE tail.  Out-matmuls trail the scores loop by two
m-tiles to stay clear of the Exp evacuations.
"""

import sys

for _p in ("/opt/trn_rl_repo", "/root/.axon_site/_ro/trn_rl_repo"):
    if _p not in sys.path:
        sys.path.append(_p)

import numpy as np
import ml_dtypes

import concourse.bacc as bacc
import concourse.mybir as mybir
import concourse.tile as tile
from concourse.bass_utils import run_bass_kernel_spmd

DT = mybir.dt.float32
FR = mybir.dt.float32r
BF = mybir.dt.bfloat16
AFT = mybir.ActivationFunctionType

B, C, HW = 4, 512, 4096          # batch, channels, tokens per batch
NQ = HW // 2                     # q tokens per core (2048)
CK = C // 128                    # contraction chunks (4)
MT = HW // 128                   # key tiles (32)
NB = NQ // 512                   # q-chunks per core (4)
SCALE = 1.0 / float(np.sqrt(C))
N_CORES = 8

_compiled = {}
_ONES = np.ones(128, dtype=np.float32)


def _build(has_bop):
    qoff = 0  # h=1 cores get a host-side token rotation instead (softmax
    # and U = P@t are invariant to a consistent key permutation)
    nc = bacc.Bacc("TRN2", target_bir_lowering=False)

    xt_e = nc.declare_dram_parameter("xt", [C, HW], BF, isOutput=False)
    xn_e = nc.declare_dram_parameter("xn", [HW, C], BF, isOutput=False)
    at_e = nc.declare_dram_parameter("at", [C, C], BF, isOutput=False)
    wvot_e = nc.declare_dram_parameter("wvot", [C, C], BF, isOutput=False)
    sbias_e = nc.declare_dram_parameter("sbias", [128, MT], DT, isOutput=False)
    ones_fr_e = nc.declare_dram_parameter("ones_fr", [128], FR, isOutput=False)
    if has_bop:
        bop_e = nc.declare_dram_parameter("bop", [C], FR, isOutput=False)
    out_e = nc.declare_dram_parameter("outT", [C, NQ], DT, isOutput=True)

    with tile.TileContext(nc) as tc:
        with (
            tc.tile_pool(name="tc", bufs=1) as tc_pool,
            tc.tile_pool(name="xn", bufs=1) as xn_pool,
            tc.tile_pool(name="wt", bufs=1) as w_pool,
            tc.tile_pool(name="consts", bufs=1) as c_pool,
            tc.tile_pool(name="qcp", bufs=2) as qc_pool,
            tc.tile_pool(name="pexp", bufs=6) as pe_pool,
            tc.tile_pool(name="accp", bufs=2) as acc_pool,
            tc.tile_pool(name="up", bufs=2) as u_pool,
            tc.tile_pool(name="rinvp", bufs=2) as rinv_pool,
            tc.tile_pool(name="srp", bufs=2) as sr_pool,
            tc.tile_pool(name="outp", bufs=5) as oc_pool,
            tc.tile_pool(name="psg", bufs=4, space="PSUM") as ps_gen,
            tc.tile_pool(name="psu", bufs=1, space="PSUM") as ps_ut,
        ):
            tc_sb = [tc_pool.tile([128, HW], BF, tag=f"t{i}", name=f"t{i}") for i in range(CK)]
            xn_sb = [xn_pool.tile([128, C], BF, tag=f"x{i}", name=f"x{i}") for i in range(MT)]
            at_sb = [w_pool.tile([128, C], BF, tag=f"a{i}", name=f"a{i}") for i in range(CK)]
            wv_sb = [w_pool.tile([128, C], BF, tag=f"w{i}", name=f"w{i}") for i in range(CK)]
            sbias_t = c_pool.tile([128, MT], DT, tag="sb", name="sbias_t")
            ones_col_r = c_pool.tile([128, 1], FR, tag="onescr", name="ones_col_r")
            ones_row_r = c_pool.tile([1, 128], FR, tag="onesrr", name="ones_row_r")
            if has_bop:
                bop_row = c_pool.tile([1, C], FR, tag="bop", name="bop_row")

            # ---- DMA issue order == consumption order ----
            for i in range(CK):
                nc.sync.dma_start(at_sb[i][:], at_e[i * 128:(i + 1) * 128, :])
            # first tc column group: the one holding chunk-0's queries
            jq = qoff // 512
            jorder = [jq] + [j for j in range(HW // 512) if j != jq]
            for ji, j in enumerate(jorder):
                for i in range(CK):
                    nc.sync.dma_start(
                        tc_sb[i][:, j * 512:(j + 1) * 512],
                        xt_e[i * 128:(i + 1) * 128, j * 512:(j + 1) * 512],
                    )
                if ji == 0:
                    for i in range(CK):
                        nc.sync.dma_start(wv_sb[i][:], wvot_e[i * 128:(i + 1) * 128, :])
                    nc.sync.dma_start(sbias_t[:], sbias_e[:, :])
                    nc.sync.dma_start(ones_col_r[:, 0:1], ones_fr_e[:])
                    nc.sync.dma_start(ones_row_r[0:1, :], ones_fr_e[:])
                    if has_bop:
                        nc.sync.dma_start(bop_row[0:1, :], bop_e[:])
            for m in range(MT):
                nc.gpsimd.dma_start(xn_sb[m][:], xn_e[m * 128:(m + 1) * 128, :])

            def emit_qproj(nb):
                qcs = [None] * CK
                for w in range(2):
                    pqs = []
                    for co in (2 * w, 2 * w + 1):
                        pq = ps_gen.tile([128, 512], DT, tag="g", name="pq")
                        for ci in range(CK):
                            nc.tensor.matmul(
                                pq[:], at_sb[ci][:, co * 128:(co + 1) * 128],
                                tc_sb[ci][:, qoff + nb * 512:qoff + (nb + 1) * 512],
                                start=(ci == 0), stop=(ci == CK - 1),
                            )
                        pqs.append(pq)
                    for pq, co in zip(pqs, (2 * w, 2 * w + 1)):
                        qc = qc_pool.tile([128, 512], BF, tag=f"qc{co}", name=f"qc{co}")
                        nc.vector.tensor_copy(qc[:], pq[:])
                        qcs[co] = qc
                return qcs

            def emit_tail(tnb, acc, u_sbs):
                # rowsum -> broadcast -> reciprocal -> U-projection -> store
                rs = ps_gen.tile([1, 512], DT, tag="g", name="rs")
                nc.tensor.matmul(rs[:], ones_col_r[:, 0:1], acc[:], start=True, stop=True)
                rs_row = sr_pool.tile([1, 512], FR, tag="rsrow", name="rs_row")
                nc.scalar.activation(rs_row[:], rs[:], AFT.Copy)
                rbc = ps_gen.tile([128, 512], DT, tag="g", name="rbc")
                nc.tensor.matmul(rbc[:], ones_row_r[0:1, :], rs_row[0:1, :],
                                 start=True, stop=True)
                rinv = rinv_pool.tile([128, 512], DT, tag="rinv", name="rinv")
                nc.vector.reciprocal_approx_fast(out=rinv[:], in_=rbc[:])
                for co in range(CK):
                    ot = ps_gen.tile([128, 512], DT, tag="g", name="ot")
                    for ci in range(CK):
                        nc.tensor.matmul(
                            ot[:], wv_sb[ci][:, co * 128:(co + 1) * 128],
                            u_sbs[ci][:],
                            start=(ci == 0),
                            stop=(ci == CK - 1) and not has_bop,
                        )
                    if has_bop:
                        nc.tensor.matmul(
                            ot[:], bop_row[0:1, co * 128:(co + 1) * 128],
                            rs_row[0:1, :], start=False, stop=True,
                            skip_group_check=True,
                        )
                    oc = oc_pool.tile([128, 512], DT, tag="oc", name="oc", bufs=5)
                    nc.vector.tensor_mul(oc[:], ot[:], rinv[:])
                    nc.sync.dma_start(
                        out_e[co * 128:(co + 1) * 128, tnb * 512:(tnb + 1) * 512], oc[:]
                    )

            prev = None
            for nb in range(NB):
                qcs = emit_qproj(nb)
                if prev is not None:
                    emit_tail(*prev)

                acc = acc_pool.tile([128, 512], FR, tag="acc", name="acc")
                uts = [ps_ut.tile([128, 512], DT, tag=f"ut{co}", name=f"ut{co}") for co in range(CK)]
                pexps = {}

                def emit_out(m):
                    pe = pexps.pop(m)
                    for co in range(CK):
                        nc.tensor.matmul(
                            uts[co][:], xn_sb[m][:, co * 128:(co + 1) * 128],
                            pe[:], start=(m == 0), stop=(m == MT - 1),
                            skip_group_check=True,
                        )

                for mt in range(MT):
                    st = ps_gen.tile([128, 512], DT, tag="g", name="st")
                    for ci in range(CK):
                        nc.tensor.matmul(
                            st[:], tc_sb[ci][:, mt * 128:(mt + 1) * 128],
                            qcs[ci][:], start=(ci == 0), stop=(ci == CK - 1),
                        )
                    pe = pe_pool.tile([128, 512], BF, tag="pe", name="pexp")
                    nc.scalar.activation(pe[:], st[:], AFT.Exp,
                                         bias=sbias_t[:, mt:mt + 1], scale=SCALE)
                    if mt == 0:
                        nc.vector.tensor_copy(acc[:], pe[:])
                    else:
                        nc.vector.tensor_add(acc[:], acc[:], pe[:])
                    pexps[mt] = pe
                    if mt >= 2:
                        emit_out(mt - 2)
                emit_out(MT - 2)
                emit_out(MT - 1)

                u_sbs = []
                for ci in range(CK):
                    u = u_pool.tile([128, 512], BF, tag=f"u{ci}", name=f"u{ci}")
                    nc.scalar.activation(u[:], uts[ci][:], AFT.Copy)
                    u_sbs.append(u)
                prev = (nb, acc, u_sbs)

            emit_tail(*prev)

    nc.compile()
    return nc


def _get_compiled(has_bop=False):
    if has_bop not in _compiled:
        _compiled[has_bop] = _build(has_bop)
    return _compiled[has_bop]


def kernel(**inputs):
    x = np.ascontiguousarray(np.asarray(inputs["x"], dtype=np.float32))
    wq = np.asarray(inputs["Wq"], dtype=np.float32)
    wk = np.asarray(inputs["Wk"], dtype=np.float32)
    wv = np.asarray(inputs["Wv"], dtype=np.float32)
    wo = np.asarray(inputs["Wo"], dtype=np.float32)
    bq = np.asarray(inputs["bq"], dtype=np.float32)
    bv = np.asarray(inputs["bv"], dtype=np.float32)
    bo = np.asarray(inputs["bo"], dtype=np.float32)

    at = np.ascontiguousarray((wq.T @ wk).astype(ml_dtypes.bfloat16))
    wvot = np.ascontiguousarray((wo @ wv).T.astype(ml_dtypes.bfloat16))
    bop = wo @ bv + bo
    has_bop = bool(np.any(bop != 0.0))
    bop_fr = np.ascontiguousarray(bop.astype(np.float32))

    xb = x.reshape(B, C, HW)
    xt_bf = xb.astype(ml_dtypes.bfloat16)
    # per-key score bias from bq (zero when bq == 0), pre-scaled
    rrow = (SCALE * ((bq @ wk) @ xb)).astype(np.float32)  # (B, HW)

    in_maps = []
    for core in range(N_CORES):
        bi, h = core // 2, core % 2
        if h == 0:
            xt_c, r_c = xt_bf[bi], rrow[bi]
        else:
            # rotate the token axis so this core's queries sit at offset 0;
            # key order is consistently permuted everywhere (softmax and
            # U = P@t are invariant to that)
            xt_c = np.concatenate([xt_bf[bi][:, NQ:], xt_bf[bi][:, :NQ]], axis=1)
            r_c = np.concatenate([rrow[bi][NQ:], rrow[bi][:NQ]])
        m = {
            "xt": np.ascontiguousarray(xt_c),
            "xn": np.ascontiguousarray(xt_c.T),
            "at": at, "wvot": wvot,
            "sbias": np.ascontiguousarray(r_c.reshape(MT, 128).T),
            "ones_fr": _ONES,
        }
        if has_bop:
            m["bop"] = bop_fr
        in_maps.append(m)

    nc = _get_compiled(has_bop)
    res = run_bass_kernel_spmd(nc, in_maps, core_ids=list(range(N_CORES)))

    out = np.empty((B, HW, C), dtype=np.float32)
    for core in range(N_CORES):
        bi, h = core // 2, core % 2
        out[bi, h * NQ:(h + 1) * NQ, :] = res.results[core]["outT"].T
    return out.reshape(B, C, 64, 64)
